# revision 28
# baseline (speedup 1.0000x reference)
"""MoE-routed conv kernel (Channel_Embedding ablation) for 8 trn2 NeuronCores.

Math (see reference):
  gates  = top2-renormalized softmax( x[:, :, -6:-1].reshape(B, D*5) @ w_gate )
  h      = tanh(conv1d(x, conv1_w, VALID) + conv1_b)            # [B, OC, L-2]
  out    = conv1d(h, conv2_w, 1x1) + conv2_b                    # [B, OC*E, L-2]
  y[b,oc,t] = sum_e gates[b,e] * out[b, oc*E+e, t]

Key algebraic fold: the expert combine commutes with the 1x1 conv, so per
batch element
  W_eff[b][oc, ic] = sum_e gates[b,e] * conv2_w[oc*E+e, ic, 0]
  b_eff[b][oc]     = sum_e gates[b,e] * conv2_b[oc*E+e]
  y[b] = W_eff[b] @ h[b] + b_eff[b]

Sharding: data-parallel over batch B=32 across 8 cores (4 each); weights
replicated.

Layout (all 128 partitions, bf16 hot path): x ships from host as bf16;
xf[64q + d, 4096p + c] = x[2q+p, d, c]. Conv matmuls are bf16 with
block-diag weights over q (K=128 = 2 batches x 64 ch); pair p=1 writes
PSUM partitions 64:128 via the matmul tile position, so each 1024-col
tile accumulates ONE [128, 1024] PSUM image covering all 4 batches ->
one tanh, block-diag combine matmuls, bias-add drains. Gating is strict
fp32 (top-2 expert selection must match the reference); its 5-column x
window rides inside the fp32 const image.

Schedule (~36us mean, vs 40-42us baseline): W_eff is emitted EARLY
(right after gating) so its DRAM-bounce DMAs clear the scalar queue
before the tanh/combine phase. x arrives in chunks [0,514), [514,1538),
[1538,2562) on sync (q=0) / gpsimd (q=1) plus a [2562,4096) tail on
scalar; the +2-shifted boundaries let a conv tile start as soon as its
own chunks land, and the small first chunk pulls the conv start
earlier. y stores all ride the sync+scalar HWDGE queues (gpsimd SWDGE
would burn ~2.5us of engine time per store job generating descriptors).
Run-to-run variance on identical code is +-3us (power/thermal state);
judge changes by multi-run means.

HARD-LEARNED CONSTRAINTS (hardware wedges or regressions observed):
 - Never queue more than ~3 bulk DMA instructions (~384 descriptors) on
   one DGE queue at once: 5 x-chunk jobs per queue wedges the device
   (NRT_EXEC_UNIT_UNRECOVERABLE).
 - gpsimd SWDGE burns ~2-3us of engine time per dma_start generating
   descriptors; HWDGE (sync/scalar) jobs start in ~1.5us.
 - The TileContext scheduler reorders instructions; fine-grained
   interleavings (per-512 conv/combine/store pipelining, per-(q,p) x
   jobs, batch-ordered 2D stores) all measured SLOWER than this
   coarse-phase schedule (38-45us vs 35.7us).
 - SBUF->SBUF DMA with a partition-split destination AP compiles but
   returns garbage on hardware; the W_eff redistribution must bounce
   through DRAM.
"""

from contextlib import ExitStack

import numpy as np

import concourse.bacc as bacc
import concourse.mybir as mybir
import concourse.tile as tile
from concourse import bass_utils

B, D, L = 32, 64, 4096
E, TOPK, OC = 8, 2, 32
LP = L - 2  # 4094 valid conv outputs
NCORES = 8
NB = B // NCORES  # batch elements per core
TS = 512  # position tile (one PSUM bank of fp32)
NT = (LP + TS - 1) // TS

FAST_DT = mybir.dt.float32r  # fp32 bits, 1 cycle/row on PE at N>=256
BF16 = mybir.dt.bfloat16

C_WG = 0  # rows 0:64 AND 64:128, [*, 40], col = t*8 + e
C_B1P = C_WG + 5 * E  # [128, 1] conv1 bias tiled 4x (partition p,q,oc)
C_GW = C_B1P + 1  # [128, 10] gwin[64q+d, 5p+t] = x[2q+p, d, L-6+t]
NCF = C_GW + 10
NW1T = 3 * 2 * OC  # bf16 image: block-diag conv1 weights [128, 192]
C2_W, C2_B, NC2 = 0, OC * OC, OC * OC + OC

_CACHE: dict = {}


def _softmax_top2(nc, sm, lg, f32, AX, OP, AF, q):
    e_sb = sm.tile([2, E], f32, name=f"e_sb{q}")
    nc.scalar.activation(e_sb[:], lg, AF.Exp)
    m1 = sm.tile([2, 1], f32, name=f"m1_{q}")
    nc.vector.reduce_max(m1[:], e_sb[:], axis=AX.X)
    lt = sm.tile([2, E], f32, name=f"lt{q}")
    nc.vector.tensor_scalar(lt[:], e_sb[:], m1[:], None, op0=OP.is_lt)
    emsk = sm.tile([2, E], f32, name=f"emsk{q}")
    nc.vector.tensor_mul(emsk[:], lt[:], e_sb[:])
    m2 = sm.tile([2, 1], f32, name=f"m2_{q}")
    nc.vector.reduce_max(m2[:], emsk[:], axis=AX.X)
    den3 = sm.tile([2, 1], f32, name=f"den3{q}")
    nc.vector.tensor_add(den3[:], m1[:], m2[:])
    rcp = sm.tile([2, 1], f32, name=f"rcp{q}")
    nc.vector.reciprocal(rcp[:], den3[:])
    ge = sm.tile([2, E], f32, name=f"ge{q}")
    nc.vector.tensor_scalar(ge[:], e_sb[:], m2[:], None, op0=OP.is_ge)
    gnum = sm.tile([2, E], f32, name=f"gnum{q}")
    nc.vector.tensor_mul(gnum[:], ge[:], e_sb[:])
    gpad = sm.tile([32, 32], f32, name=f"gpad{q}")
    nc.vector.memset(gpad[:], 0.0)
    nc.vector.tensor_scalar(gpad[0:2, 0:E], gnum[:], rcp[:], None, op0=OP.mult)
    gtr = sm.tile([32, 32], f32, name=f"gtr{q}")
    nc.vector.transpose(gtr[:], gpad[:])
    return gtr


def _emit(ctx, tc, nc, x_d, cf_d, w1_d, c2r_d, y_d):
    f32 = mybir.dt.float32
    AF = mybir.ActivationFunctionType
    AX = mybir.AxisListType
    OP = mybir.AluOpType

    const = ctx.enter_context(tc.tile_pool(name="const", bufs=1))
    sm = ctx.enter_context(tc.tile_pool(name="sm", bufs=1))
    hsb = ctx.enter_context(tc.tile_pool(name="hsb", bufs=3))
    psum_h = ctx.enter_context(tc.tile_pool(name="ph", bufs=3, space="PSUM"))
    psum_o = ctx.enter_context(tc.tile_pool(name="po", bufs=2, space="PSUM"))
    dram = ctx.enter_context(tc.tile_pool(name="dram", bufs=1, space="DRAM"))

    cf = const.tile([128, NCF], f32)
    nc.scalar.dma_start(cf[:], cf_d.ap(), max_dma_last_dim=NCF)
    w1t = const.tile([128, NW1T], BF16)
    nc.scalar.dma_start(w1t[:], w1_d.ap(), max_dma_last_dim=NW1T)
    c2r = const.tile([E, NC2], FAST_DT)
    nc.scalar.dma_start(c2r[:], c2r_d.ap().bitcast(FAST_DT), max_dma_last_dim=NC2)
    c2w = c2r[0:E, C2_W : C2_W + OC * OC]
    c2b = c2r[0:E, C2_B : C2_B + OC]
    b1p = cf[:, C_B1P : C_B1P + 1]

    xf = const.tile([2 * D, 2 * L], BF16)
    xv = x_d.ap().rearrange("(q p) d c -> q d p c", q=2)

    def load_chunk_q(eng, q, a0, a1):
        eng.dma_start(
            xf[D * q : D * q + D, :].rearrange("d (p c) -> d p c", p=2)[
                :, :, a0:a1
            ],
            xv[q : q + 1, :, :, a0:a1],
        )

    CHB = [0, 514, 1538, 2562]
    for j in range(len(CHB) - 1):
        load_chunk_q(nc.sync, 0, CHB[j], CHB[j + 1])
    for j in range(len(CHB) - 1):
        load_chunk_q(nc.gpsimd, 1, CHB[j], CHB[j + 1])
    load_chunk_q(nc.scalar, 0, 2562, 4096)
    load_chunk_q(nc.scalar, 1, 2562, 4096)

    warm = sm.tile([1, 8], f32)
    nc.vector.memset(warm[:], 0.0)
    warm2 = sm.tile([1, 8], f32)
    nc.scalar.activation(warm2[:], warm[:], AF.Exp)

    wsrc = sm.tile([128, 512], BF16)
    nc.vector.memset(wsrc[:], 0.0)
    wupt = psum_h.tile([128, 1024], f32, tag="hp")
    wup = wupt[0:64, 0:512]
    for _ in range(3):
        nc.tensor.matmul(wup, wsrc[:, 0:64], wsrc[:], start=True, stop=True)

    gwin = cf[:, C_GW : C_GW + 10].rearrange("d (p t) -> d t p", p=2)
    gtrs = []
    for q in range(2):
        lgt = psum_o.tile([128, 512], f32, tag="op", name=f"lg{q}")
        lg = lgt[0:2, 0:E]
        for t in range(5):
            nc.tensor.matmul(
                lg,
                gwin[D * q : D * q + D, t : t + 1, :],
                cf[D * q : D * q + D, C_WG + E * t : C_WG + E * t + E],
                start=(t == 0),
                stop=(t == 4),
            )
        gtrs.append(_softmax_top2(nc, sm, lg, f32, AX, OP, AF, q))
    gT = sm.tile([E, NB], FAST_DT)
    for q in range(2):
        nc.vector.tensor_copy(
            gT[:, q : q + 3 : 2], gtrs[q][0:E, 0:2]
        )

    weT = const.tile([128, 128], BF16)
    beff = sm.tile([128, 1], f32)

    def emit_weff():
        wp3t = psum_o.tile([128, 512], f32, tag="op")
        wp3 = wp3t[0:NB, 0:OC]
        nc.tensor.matmul(wp3, gT[:], c2b[:], start=True, stop=True)
        weff2 = sm.tile([NB, OC * 33], BF16)
        wp1t = psum_o.tile([128, 512], f32, tag="op")
        wp1 = wp1t[0:NB, 0:512]
        nc.tensor.matmul(wp1, gT[:], c2w[:, 0:512], start=True, stop=True)
        nc.vector.tensor_copy(
            weff2[:, 0 : 16 * 33].rearrange("b (r s) -> b r s", s=33)[:, :, 0:32],
            wp1.rearrange("b (r s) -> b r s", s=32),
        )
        wp2t = psum_h.tile([128, 1024], f32, tag="hp")
        wp2 = wp2t[0:NB, 0:512]
        nc.tensor.matmul(wp2, gT[:], c2w[:, 512:1024], start=True, stop=True)
        nc.vector.tensor_copy(
            weff2[:, 16 * 33 :].rearrange("b (r s) -> b r s", s=33)[:, :, 0:32],
            wp2.rearrange("b (r s) -> b r s", s=32),
        )
        nc.vector.tensor_copy(
            weff2[:].rearrange("b (r s) -> b r s", s=33)[:, :, 32:33],
            wp3.rearrange("b (r s) -> b r s", s=1),
        )

        wscr = dram.tile([NB, OC * 33], BF16)
        nc.scalar.dma_start(wscr[:], weff2[:], max_dma_last_dim=OC * 33)
        wpk = const.tile([128, 33], BF16)
        nc.scalar.dma_start(
            wpk[:], wscr[:, :].rearrange("b (r s) -> (b r) s", s=33)
        )
        nc.vector.memset(weT[:], 0.0)
        for j in range(4):
            nc.vector.tensor_copy(
                weT[32 * j : 32 * j + 32, 32 * j : 32 * j + 32],
                wpk[32 * j : 32 * j + 32, 0:32],
            )
        nc.vector.tensor_copy(beff[:], wpk[:, 32:33])

    emit_weff()

    yb = const.tile([128, LP], BF16)

    TS2 = 2 * TS
    hss = []
    for it in range(4):
        c0 = it * TS2
        n = min(TS2, LP - c0)
        hp = psum_h.tile([128, TS2], f32, tag="hp")
        for h in range(2):
            b0 = c0 + TS * h
            m = min(TS, LP - b0)
            for k in range(3):
                for p in range(2):
                    nc.tensor.matmul(
                        hp[64 * p : 64 * p + 64, TS * h : TS * h + m],
                        w1t[:, 64 * k : 64 * k + 64],
                        xf[:, L * p + b0 + k : L * p + b0 + k + m],
                        start=(k == 0),
                        stop=(k == 2),
                    )
        hs = hsb.tile([128, TS2], BF16, tag="hs", name=f"hs{it}")
        nc.scalar.activation(hs[:, 0:n], hp[:, 0:n], AF.Tanh, bias=b1p, scale=1.0)
        hss.append(hs)

    for it in range(4):
        c0 = it * TS2
        for h in range(2):
            b0 = c0 + TS * h
            m = min(TS, LP - b0)
            op_ = psum_o.tile([128, TS], f32, tag="op")
            nc.tensor.matmul(
                op_[:, 0:m], weT[:], hss[it][:, TS * h : TS * h + m],
                start=True, stop=True,
            )
            nc.vector.tensor_scalar(
                yb[:, b0 : b0 + m], op_[:, 0:m], beff[:], None, op0=OP.add
            )
        if it == 1:
            _store_half(
                nc, [nc.sync, nc.sync, nc.scalar, nc.scalar], y_d, yb,
                0, LP // 2,
            )
    _store_half(nc, [nc.sync, nc.sync, nc.scalar, nc.scalar], y_d, yb, LP // 2, LP)


def _store_half(nc, engs, y_d, yb, a0, a1):
    for b in range(NB):
        p, q = b % 2, b // 2
        j = 2 * p + q
        engs[b].dma_start(
            y_d.ap()[b, :, a0:a1],
            yb[32 * j : 32 * j + 32, a0:a1],
            max_dma_last_dim=a1 - a0,
        )


def _build():
    if "nc" in _CACHE:
        return _CACHE["nc"]
    nc = bacc.Bacc(
        "TRN2",
        target_bir_lowering=False,
        debug=False,
        num_devices=NCORES,
        detect_race_conditions=False,
    )
    f32 = mybir.dt.float32
    x_d = nc.dram_tensor("x", [NB, D, L], BF16, kind="ExternalInput")
    cf_d = nc.dram_tensor("cf", [128, NCF], f32, kind="ExternalInput")
    w1_d = nc.dram_tensor("w1", [128, NW1T], BF16, kind="ExternalInput")
    c2r_d = nc.dram_tensor("c2r", [E, NC2], f32, kind="ExternalInput")
    y_d = nc.dram_tensor("y", [NB, OC, LP], BF16, kind="ExternalOutput")

    with tile.TileContext(nc) as tc:
        with ExitStack() as ctx:
            _emit(ctx, tc, nc, x_d, cf_d, w1_d, c2r_d, y_d)
    nc.compile()
    _CACHE["nc"] = nc
    return nc


def _prep_weights(w_gate, conv1_w, conv1_b, conv2_w, conv2_b):
    import ml_dtypes

    bf16 = ml_dtypes.bfloat16
    w_gate = np.asarray(w_gate, np.float32)
    conv1_w = np.asarray(conv1_w, np.float32)
    conv1_b = np.asarray(conv1_b, np.float32)
    conv2_w = np.asarray(conv2_w, np.float32)
    conv2_b = np.asarray(conv2_b, np.float32)
    w1 = np.zeros((128, NW1T), bf16)
    wkt = conv1_w.transpose(1, 2, 0).astype(bf16)  # [d, k, oc]
    for k in range(3):
        w1[0:D, 64 * k : 64 * k + OC] = wkt[:, k, :]
        w1[D : 2 * D, 64 * k + OC : 64 * k + 2 * OC] = wkt[:, k, :]
    cf = np.zeros((128, NCF), np.float32)
    wgr = w_gate.reshape(D, 5 * E)
    cf[0:D, C_WG : C_WG + 5 * E] = wgr
    cf[D : 2 * D, C_WG : C_WG + 5 * E] = wgr
    cf[:, C_B1P] = np.tile(conv1_b, 4)
    c2 = np.zeros((E, NC2), np.float32)
    c2[:, C2_W : C2_W + OC * OC] = (
        conv2_w[:, :, 0].reshape(OC, E, OC).transpose(1, 2, 0).reshape(E, OC * OC)
    )
    c2[:, C2_B : C2_B + OC] = conv2_b.reshape(OC, E).T
    return np.ascontiguousarray(w1), cf, np.ascontiguousarray(c2)


def _run(x, w_gate, conv1_w, conv1_b, conv2_w, conv2_b, **spmd_kwargs):
    import ml_dtypes

    bf16 = ml_dtypes.bfloat16
    x = np.asarray(x, np.float32)
    assert x.shape == (B, D, L), x.shape
    w1, cf, c2 = _prep_weights(w_gate, conv1_w, conv1_b, conv2_w, conv2_b)
    xb = np.ascontiguousarray(x.astype(bf16))
    nc = _build()
    in_maps = []
    for i in range(NCORES):
        xs = x[NB * i : NB * (i + 1)]
        cfi = cf.copy()
        gw = xs[:, :, L - 6 : L - 1].reshape(2, 2, D, 5)  # [q, p, d, t]
        cfi[:, C_GW : C_GW + 10] = (
            gw.transpose(0, 2, 1, 3).reshape(2 * D, 10)
        )
        in_maps.append(
            {
                "x": np.ascontiguousarray(xb[NB * i : NB * (i + 1)]),
                "cf": cfi,
                "w1": w1,
                "c2r": c2,
            }
        )
    res = bass_utils.run_bass_kernel_spmd(
        nc, in_maps, core_ids=list(range(NCORES)), **spmd_kwargs
    )
    y = np.concatenate([r["y"] for r in res.results], axis=0)
    return np.ascontiguousarray(y.astype(np.float32)), res


def kernel(x, w_gate, conv1_w, conv1_b, conv2_w, conv2_b):
    y, _ = _run(x, w_gate, conv1_w, conv1_b, conv2_w, conv2_b)
    return y


# revision 29
# speedup vs baseline: 1.0100x; 1.0100x over previous
"""MoE-routed conv kernel (Channel_Embedding ablation) for 8 trn2 NeuronCores.

Math (see reference):
  gates  = top2-renormalized softmax( x[:, :, -6:-1].reshape(B, D*5) @ w_gate )
  h      = tanh(conv1d(x, conv1_w, VALID) + conv1_b)            # [B, OC, L-2]
  out    = conv1d(h, conv2_w, 1x1) + conv2_b                    # [B, OC*E, L-2]
  y[b,oc,t] = sum_e gates[b,e] * out[b, oc*E+e, t]

Key algebraic fold: the expert combine commutes with the 1x1 conv, so per
batch element
  W_eff[b][oc, ic] = sum_e gates[b,e] * conv2_w[oc*E+e, ic, 0]
  b_eff[b][oc]     = sum_e gates[b,e] * conv2_b[oc*E+e]
  y[b] = W_eff[b] @ h[b] + b_eff[b]

Sharding: data-parallel over batch B=32 across 8 cores (4 each); weights
replicated.

Layout (all 128 partitions, bf16 hot path): x ships from host as bf16;
xf[64q + d, 4096p + c] = x[2q+p, d, c]. Conv matmuls are bf16 with
block-diag weights over q (K=128 = 2 batches x 64 ch); pair p=1 writes
PSUM partitions 64:128 via the matmul tile position, so each 1024-col
tile accumulates ONE [128, 1024] PSUM image covering all 4 batches ->
one tanh, block-diag combine matmuls, bias-add drains. Gating is strict
fp32 (top-2 expert selection must match the reference); its 5-column x
window rides inside the fp32 const image.

Schedule (~36us mean, vs 40-42us baseline): W_eff is emitted EARLY
(right after gating) so its DRAM-bounce DMAs clear the scalar queue
before the tanh/combine phase. x arrives in chunks [0,514), [514,1538),
[1538,2562) on sync (q=0) / gpsimd (q=1) plus a [2562,4096) tail on
scalar; the +2-shifted boundaries let a conv tile start as soon as its
own chunks land, and the small first chunk pulls the conv start
earlier. y stores all ride the sync+scalar HWDGE queues (gpsimd SWDGE
would burn ~2.5us of engine time per store job generating descriptors).
Run-to-run variance on identical code is +-3us (power/thermal state);
judge changes by multi-run means.

HARD-LEARNED CONSTRAINTS (hardware wedges or regressions observed):
 - Never queue more than ~3 bulk DMA instructions (~384 descriptors) on
   one DGE queue at once: 5 x-chunk jobs per queue wedges the device
   (NRT_EXEC_UNIT_UNRECOVERABLE).
 - gpsimd SWDGE burns ~2-3us of engine time per dma_start generating
   descriptors; HWDGE (sync/scalar) jobs start in ~1.5us.
 - The TileContext scheduler reorders instructions; fine-grained
   interleavings (per-512 conv/combine/store pipelining, per-(q,p) x
   jobs, batch-ordered 2D stores) all measured SLOWER than this
   coarse-phase schedule (38-45us vs 35.7us).
 - SBUF->SBUF DMA with a partition-split destination AP compiles but
   returns garbage on hardware; the W_eff redistribution must bounce
   through DRAM.
"""

from contextlib import ExitStack

import numpy as np

import concourse.bacc as bacc
import concourse.mybir as mybir
import concourse.tile as tile
from concourse import bass_utils

B, D, L = 32, 64, 4096
E, TOPK, OC = 8, 2, 32
LP = L - 2  # 4094 valid conv outputs
NCORES = 8
NB = B // NCORES  # batch elements per core
TS = 512  # position tile (one PSUM bank of fp32)
NT = (LP + TS - 1) // TS

FAST_DT = mybir.dt.float32r  # fp32 bits, 1 cycle/row on PE at N>=256
BF16 = mybir.dt.bfloat16

C_WG = 0  # rows 0:64 AND 64:128, [*, 40], col = t*8 + e
C_B1P = C_WG + 5 * E  # [128, 1] conv1 bias tiled 4x (partition p,q,oc)
C_GW = C_B1P + 1  # [128, 10] gwin[64q+d, 5p+t] = x[2q+p, d, L-6+t]
NCF = C_GW + 10
NW1T = 3 * 2 * OC  # bf16 image: block-diag conv1 weights [128, 192]
C2_W, C2_B, NC2 = 0, OC * OC, OC * OC + OC

_CACHE: dict = {}


def _softmax_top2(nc, sm, lg, f32, AX, OP, AF, q):
    e_sb = sm.tile([2, E], f32, name=f"e_sb{q}")
    nc.scalar.activation(e_sb[:], lg, AF.Exp)
    m1 = sm.tile([2, 1], f32, name=f"m1_{q}")
    nc.vector.reduce_max(m1[:], e_sb[:], axis=AX.X)
    lt = sm.tile([2, E], f32, name=f"lt{q}")
    nc.vector.tensor_scalar(lt[:], e_sb[:], m1[:], None, op0=OP.is_lt)
    emsk = sm.tile([2, E], f32, name=f"emsk{q}")
    nc.vector.tensor_mul(emsk[:], lt[:], e_sb[:])
    m2 = sm.tile([2, 1], f32, name=f"m2_{q}")
    nc.vector.reduce_max(m2[:], emsk[:], axis=AX.X)
    den3 = sm.tile([2, 1], f32, name=f"den3{q}")
    nc.vector.tensor_add(den3[:], m1[:], m2[:])
    rcp = sm.tile([2, 1], f32, name=f"rcp{q}")
    nc.vector.reciprocal(rcp[:], den3[:])
    ge = sm.tile([2, E], f32, name=f"ge{q}")
    nc.vector.tensor_scalar(ge[:], e_sb[:], m2[:], None, op0=OP.is_ge)
    gnum = sm.tile([2, E], f32, name=f"gnum{q}")
    nc.vector.tensor_mul(gnum[:], ge[:], e_sb[:])
    gpad = sm.tile([32, 32], f32, name=f"gpad{q}")
    nc.vector.memset(gpad[:], 0.0)
    nc.vector.tensor_scalar(gpad[0:2, 0:E], gnum[:], rcp[:], None, op0=OP.mult)
    gtr = sm.tile([32, 32], f32, name=f"gtr{q}")
    nc.vector.transpose(gtr[:], gpad[:])
    return gtr


def _emit(ctx, tc, nc, x_d, cf_d, w1_d, c2r_d, y_d):
    f32 = mybir.dt.float32
    AF = mybir.ActivationFunctionType
    AX = mybir.AxisListType
    OP = mybir.AluOpType

    const = ctx.enter_context(tc.tile_pool(name="const", bufs=1))
    sm = ctx.enter_context(tc.tile_pool(name="sm", bufs=1))
    # 4 bufs: all four hs tiles live at once, so tanh(3) never waits for
    # the weT-gated combines to release hs0's buffer (false WAR dep)
    hsb = ctx.enter_context(tc.tile_pool(name="hsb", bufs=4))
    psum_h = ctx.enter_context(tc.tile_pool(name="ph", bufs=3, space="PSUM"))
    psum_o = ctx.enter_context(tc.tile_pool(name="po", bufs=2, space="PSUM"))
    dram = ctx.enter_context(tc.tile_pool(name="dram", bufs=1, space="DRAM"))

    cf = const.tile([128, NCF], f32)
    nc.scalar.dma_start(cf[:], cf_d.ap(), max_dma_last_dim=NCF)
    w1t = const.tile([128, NW1T], BF16)
    nc.scalar.dma_start(w1t[:], w1_d.ap(), max_dma_last_dim=NW1T)
    c2r = const.tile([E, NC2], FAST_DT)
    nc.scalar.dma_start(c2r[:], c2r_d.ap().bitcast(FAST_DT), max_dma_last_dim=NC2)
    c2w = c2r[0:E, C2_W : C2_W + OC * OC]
    c2b = c2r[0:E, C2_B : C2_B + OC]
    b1p = cf[:, C_B1P : C_B1P + 1]

    xf = const.tile([2 * D, 2 * L], BF16)
    xv = x_d.ap().rearrange("(q p) d c -> q d p c", q=2)

    def load_chunk_q(eng, q, a0, a1):
        eng.dma_start(
            xf[D * q : D * q + D, :].rearrange("d (p c) -> d p c", p=2)[
                :, :, a0:a1
            ],
            xv[q : q + 1, :, :, a0:a1],
        )

    CHB = [0, 514, 1538, 2562]
    for j in range(len(CHB) - 1):
        load_chunk_q(nc.sync, 0, CHB[j], CHB[j + 1])
    for j in range(len(CHB) - 1):
        load_chunk_q(nc.gpsimd, 1, CHB[j], CHB[j + 1])
    load_chunk_q(nc.scalar, 0, 2562, 4096)
    load_chunk_q(nc.scalar, 1, 2562, 4096)

    warm = sm.tile([1, 8], f32)
    nc.vector.memset(warm[:], 0.0)
    warm2 = sm.tile([1, 8], f32)
    nc.scalar.activation(warm2[:], warm[:], AF.Exp)

    wsrc = sm.tile([128, 512], BF16)
    nc.vector.memset(wsrc[:], 0.0)
    wupt = psum_h.tile([128, 1024], f32, tag="hp")
    wup = wupt[0:64, 0:512]
    for _ in range(3):
        nc.tensor.matmul(wup, wsrc[:, 0:64], wsrc[:], start=True, stop=True)

    gwin = cf[:, C_GW : C_GW + 10].rearrange("d (p t) -> d t p", p=2)
    gtrs = []
    for q in range(2):
        lgt = psum_o.tile([128, 512], f32, tag="op", name=f"lg{q}")
        lg = lgt[0:2, 0:E]
        for t in range(5):
            nc.tensor.matmul(
                lg,
                gwin[D * q : D * q + D, t : t + 1, :],
                cf[D * q : D * q + D, C_WG + E * t : C_WG + E * t + E],
                start=(t == 0),
                stop=(t == 4),
            )
        gtrs.append(_softmax_top2(nc, sm, lg, f32, AX, OP, AF, q))
    gT = sm.tile([E, NB], FAST_DT)
    for q in range(2):
        nc.vector.tensor_copy(
            gT[:, q : q + 3 : 2], gtrs[q][0:E, 0:2]
        )

    weT = const.tile([128, 128], BF16)
    beff = sm.tile([128, 1], f32)

    def emit_weff():
        wp3t = psum_o.tile([128, 512], f32, tag="op")
        wp3 = wp3t[0:NB, 0:OC]
        nc.tensor.matmul(wp3, gT[:], c2b[:], start=True, stop=True)
        weff2 = sm.tile([NB, OC * 33], BF16)
        wp1t = psum_o.tile([128, 512], f32, tag="op")
        wp1 = wp1t[0:NB, 0:512]
        nc.tensor.matmul(wp1, gT[:], c2w[:, 0:512], start=True, stop=True)
        nc.vector.tensor_copy(
            weff2[:, 0 : 16 * 33].rearrange("b (r s) -> b r s", s=33)[:, :, 0:32],
            wp1.rearrange("b (r s) -> b r s", s=32),
        )
        wp2t = psum_h.tile([128, 1024], f32, tag="hp")
        wp2 = wp2t[0:NB, 0:512]
        nc.tensor.matmul(wp2, gT[:], c2w[:, 512:1024], start=True, stop=True)
        nc.vector.tensor_copy(
            weff2[:, 16 * 33 :].rearrange("b (r s) -> b r s", s=33)[:, :, 0:32],
            wp2.rearrange("b (r s) -> b r s", s=32),
        )
        nc.vector.tensor_copy(
            weff2[:].rearrange("b (r s) -> b r s", s=33)[:, :, 32:33],
            wp3.rearrange("b (r s) -> b r s", s=1),
        )

        wscr = dram.tile([NB, OC * 33], BF16)
        nc.scalar.dma_start(wscr[:], weff2[:], max_dma_last_dim=OC * 33)
        wpk = const.tile([128, 33], BF16)
        nc.scalar.dma_start(
            wpk[:], wscr[:, :].rearrange("b (r s) -> (b r) s", s=33)
        )
        nc.vector.memset(weT[:], 0.0)
        for j in range(4):
            nc.vector.tensor_copy(
                weT[32 * j : 32 * j + 32, 32 * j : 32 * j + 32],
                wpk[32 * j : 32 * j + 32, 0:32],
            )
        nc.vector.tensor_copy(beff[:], wpk[:, 32:33])

    emit_weff()

    yb = const.tile([128, LP], BF16)

    TS2 = 2 * TS
    hss = []
    for it in range(4):
        c0 = it * TS2
        n = min(TS2, LP - c0)
        hp = psum_h.tile([128, TS2], f32, tag="hp")
        for h in range(2):
            b0 = c0 + TS * h
            m = min(TS, LP - b0)
            for k in range(3):
                for p in range(2):
                    nc.tensor.matmul(
                        hp[64 * p : 64 * p + 64, TS * h : TS * h + m],
                        w1t[:, 64 * k : 64 * k + 64],
                        xf[:, L * p + b0 + k : L * p + b0 + k + m],
                        start=(k == 0),
                        stop=(k == 2),
                    )
        hs = hsb.tile([128, TS2], BF16, tag="hs", name=f"hs{it}")
        nc.scalar.activation(hs[:, 0:n], hp[:, 0:n], AF.Tanh, bias=b1p, scale=1.0)
        hss.append(hs)

    for it in range(4):
        c0 = it * TS2
        for h in range(2):
            b0 = c0 + TS * h
            m = min(TS, LP - b0)
            op_ = psum_o.tile([128, TS], f32, tag="op")
            nc.tensor.matmul(
                op_[:, 0:m], weT[:], hss[it][:, TS * h : TS * h + m],
                start=True, stop=True,
            )
            nc.vector.tensor_scalar(
                yb[:, b0 : b0 + m], op_[:, 0:m], beff[:], None, op0=OP.add
            )
        if it == 1:
            _store_half(
                nc, [nc.sync, nc.sync, nc.scalar, nc.scalar], y_d, yb,
                0, LP // 2,
            )
    _store_half(nc, [nc.sync, nc.sync, nc.scalar, nc.scalar], y_d, yb, LP // 2, LP)


def _store_half(nc, engs, y_d, yb, a0, a1):
    for b in range(NB):
        p, q = b % 2, b // 2
        j = 2 * p + q
        engs[b].dma_start(
            y_d.ap()[b, :, a0:a1],
            yb[32 * j : 32 * j + 32, a0:a1],
            max_dma_last_dim=a1 - a0,
        )


def _build():
    if "nc" in _CACHE:
        return _CACHE["nc"]
    nc = bacc.Bacc(
        "TRN2",
        target_bir_lowering=False,
        debug=False,
        num_devices=NCORES,
        detect_race_conditions=False,
    )
    f32 = mybir.dt.float32
    x_d = nc.dram_tensor("x", [NB, D, L], BF16, kind="ExternalInput")
    cf_d = nc.dram_tensor("cf", [128, NCF], f32, kind="ExternalInput")
    w1_d = nc.dram_tensor("w1", [128, NW1T], BF16, kind="ExternalInput")
    c2r_d = nc.dram_tensor("c2r", [E, NC2], f32, kind="ExternalInput")
    y_d = nc.dram_tensor("y", [NB, OC, LP], BF16, kind="ExternalOutput")

    with tile.TileContext(nc) as tc:
        with ExitStack() as ctx:
            _emit(ctx, tc, nc, x_d, cf_d, w1_d, c2r_d, y_d)
    nc.compile()
    _CACHE["nc"] = nc
    return nc


def _prep_weights(w_gate, conv1_w, conv1_b, conv2_w, conv2_b):
    import ml_dtypes

    bf16 = ml_dtypes.bfloat16
    w_gate = np.asarray(w_gate, np.float32)
    conv1_w = np.asarray(conv1_w, np.float32)
    conv1_b = np.asarray(conv1_b, np.float32)
    conv2_w = np.asarray(conv2_w, np.float32)
    conv2_b = np.asarray(conv2_b, np.float32)
    w1 = np.zeros((128, NW1T), bf16)
    wkt = conv1_w.transpose(1, 2, 0).astype(bf16)  # [d, k, oc]
    for k in range(3):
        w1[0:D, 64 * k : 64 * k + OC] = wkt[:, k, :]
        w1[D : 2 * D, 64 * k + OC : 64 * k + 2 * OC] = wkt[:, k, :]
    cf = np.zeros((128, NCF), np.float32)
    wgr = w_gate.reshape(D, 5 * E)
    cf[0:D, C_WG : C_WG + 5 * E] = wgr
    cf[D : 2 * D, C_WG : C_WG + 5 * E] = wgr
    cf[:, C_B1P] = np.tile(conv1_b, 4)
    c2 = np.zeros((E, NC2), np.float32)
    c2[:, C2_W : C2_W + OC * OC] = (
        conv2_w[:, :, 0].reshape(OC, E, OC).transpose(1, 2, 0).reshape(E, OC * OC)
    )
    c2[:, C2_B : C2_B + OC] = conv2_b.reshape(OC, E).T
    return np.ascontiguousarray(w1), cf, np.ascontiguousarray(c2)


def _run(x, w_gate, conv1_w, conv1_b, conv2_w, conv2_b, **spmd_kwargs):
    import ml_dtypes

    bf16 = ml_dtypes.bfloat16
    x = np.asarray(x, np.float32)
    assert x.shape == (B, D, L), x.shape
    w1, cf, c2 = _prep_weights(w_gate, conv1_w, conv1_b, conv2_w, conv2_b)
    xb = np.ascontiguousarray(x.astype(bf16))
    nc = _build()
    in_maps = []
    for i in range(NCORES):
        xs = x[NB * i : NB * (i + 1)]
        cfi = cf.copy()
        gw = xs[:, :, L - 6 : L - 1].reshape(2, 2, D, 5)  # [q, p, d, t]
        cfi[:, C_GW : C_GW + 10] = (
            gw.transpose(0, 2, 1, 3).reshape(2 * D, 10)
        )
        in_maps.append(
            {
                "x": np.ascontiguousarray(xb[NB * i : NB * (i + 1)]),
                "cf": cfi,
                "w1": w1,
                "c2r": c2,
            }
        )
    res = bass_utils.run_bass_kernel_spmd(
        nc, in_maps, core_ids=list(range(NCORES)), **spmd_kwargs
    )
    y = np.concatenate([r["y"] for r in res.results], axis=0)
    return np.ascontiguousarray(y.astype(np.float32)), res


def kernel(x, w_gate, conv1_w, conv1_b, conv2_w, conv2_b):
    y, _ = _run(x, w_gate, conv1_w, conv1_b, conv2_w, conv2_b)
    return y
